# revision 3
# baseline (speedup 1.0000x reference)
# Trainium2 Bass kernel for AttentionWithSink
# B=2, S=2048, D=1024, H=16 heads (hd=64), 8 sink tokens, full bidirectional
# attention over T=2056 tokens, output projection back to D.
#
# Sharding: 8 cores = 2 batches x 4 head-groups (4 heads each).
# Each core computes QKV for its 4 heads over its batch, transposed-scores
# attention (keys on partitions => no transposes anywhere), and a partial
# output projection over its 256 head-dims. Host sums the 4 partials per
# batch (tensor-parallel unshard).
import numpy as np

B, S, D, H, HD, NS = 2, 2048, 1024, 16, 64, 8
T = S + NS            # 2056 tokens incl. sinks (sinks stored LAST)
NCORES = 8
HPG = 4               # heads per group/core
GD = HPG * HD         # 256 head-dims per core
NKC = 17              # key chunks: 16*128 + 8
KREM = T - 16 * 128   # 8
NQC = 4               # query chunks
QCH = 512
VBLK = 130            # v' block: vA(64)|1|vB(64)|1

_prog_cache = {}


def _build_program():
    import concourse.bass as bass  # noqa: F401
    import concourse.mybir as mybir
    import concourse.tile as tile
    from concourse import bacc

    F32R = mybir.dt.float32r
    F32 = mybir.dt.float32
    AF = mybir.ActivationFunctionType
    ALU = mybir.AluOpType

    nc = bacc.Bacc("TRN2", num_devices=NCORES)
    xT_d = nc.dram_tensor("xT", [D, T], F32R, kind="ExternalInput")
    wq_d = nc.dram_tensor("wq_t", [D, GD], F32R, kind="ExternalInput")
    wk_d = nc.dram_tensor("wk_t", [D, GD], F32R, kind="ExternalInput")
    wv_d = nc.dram_tensor("wv_t", [D, GD], F32R, kind="ExternalInput")
    wo_d = nc.dram_tensor("wo_t", [GD, D], F32R, kind="ExternalInput")
    bq_d = nc.dram_tensor("bq", [GD, 1], F32, kind="ExternalInput")
    bk_d = nc.dram_tensor("bk", [GD, 1], F32, kind="ExternalInput")
    bv_d = nc.dram_tensor("bv", [1, GD], F32, kind="ExternalInput")
    bo_d = nc.dram_tensor("bo", [1, D], F32, kind="ExternalInput")
    y_d = nc.dram_tensor("y", [S, D], F32, kind="ExternalOutput")

    with tile.TileContext(nc) as tc:
        with (
            tc.tile_pool(name="persist", bufs=1) as pp,
            tc.tile_pool(name="pt", bufs=3) as ptp,
            tc.tile_pool(name="onT", bufs=3) as onp,
            tc.tile_pool(name="ysb", bufs=3) as yp,
            tc.tile_pool(name="small", bufs=2) as sp,
        ):
            qT = [pp.tile([128, S], F32R, tag=f"qT{i}", name=f"qT{i}") for i in range(2)]
            kT = [pp.tile([128, T], F32R, tag=f"kT{i}", name=f"kT{i}") for i in range(2)]
            vp = [pp.tile([128, NKC * VBLK], F32R, tag=f"vp{i}", name=f"vp{i}") for i in range(2)]
            wo_sb = [pp.tile([128, D], F32R, tag=f"wo{i}", name=f"wo{i}") for i in range(2)]
            bqt = [pp.tile([128, 1], F32, tag=f"bq{i}", name=f"bq{i}") for i in range(2)]
            bkt = [pp.tile([128, 1], F32, tag=f"bk{i}", name=f"bk{i}") for i in range(2)]
            for i in range(2):
                nc.sync.dma_start(wo_sb[i][:], wo_d[i * 128 : (i + 1) * 128, :])
                nc.sync.dma_start(bqt[i][:], bq_d[i * 128 : (i + 1) * 128, :])
                nc.sync.dma_start(bkt[i][:], bk_d[i * 128 : (i + 1) * 128, :])
            bv_sb = pp.tile([1, GD], F32, tag="bv")
            bo_sb = pp.tile([1, D], F32, tag="bo")
            nc.sync.dma_start(bv_sb[:], bv_d[:])
            nc.sync.dma_start(bo_sb[:], bo_d[:])
            bv_bc = pp.tile([128, GD], F32, tag="bvbc")
            bo_bc = pp.tile([128, D], F32, tag="bobc")
            nc.gpsimd.partition_broadcast(bv_bc[:], bv_sb[:])
            nc.gpsimd.partition_broadcast(bo_bc[:], bo_sb[:])
            # ones columns of v' (value columns overwritten below)
            for i in range(2):
                nc.vector.memset(vp[i][:].bitcast(F32), 1.0)

            # ---------------- Phase B: QKV projections ----------------
            with (
                tc.tile_pool(name="xw", bufs=1) as xw,
                tc.tile_pool(name="psqk", bufs=2, space="PSUM") as psb,
                tc.tile_pool(name="psv", bufs=2, space="PSUM") as psv,
            ):
                xt = [xw.tile([128, T], F32R, tag=f"x{dd}", name=f"x{dd}") for dd in range(8)]
                for dd in range(8):
                    nc.sync.dma_start(xt[dd][:], xT_d[dd * 128 : (dd + 1) * 128, :])
                w_sb = {}
                for nm, dram in (("q", wq_d), ("k", wk_d), ("v", wv_d)):
                    w = xw.tile([128, 8 * GD], F32R, tag=f"w{nm}", name=f"w{nm}")
                    for dd in range(8):
                        nc.sync.dma_start(
                            w[:, dd * GD : (dd + 1) * GD],
                            dram[dd * 128 : (dd + 1) * 128, :],
                        )
                    w_sb[nm] = w

                # v natural layout [tokens, hd] in v' blocks with ones cols
                for tcx in range(NKC):
                    kk = 128 if tcx < 16 else KREM
                    pv = psv.tile([128, GD], F32, tag="pv")
                    for dd in range(8):
                        nc.tensor.matmul(
                            pv[:kk, :],
                            xt[dd][:, tcx * 128 : tcx * 128 + kk],
                            w_sb["v"][:, dd * GD : (dd + 1) * GD],
                            start=(dd == 0),
                            stop=(dd == 7),
                        )
                    c0 = tcx * VBLK
                    for pr in range(2):
                        for hh in range(2):
                            h = pr * 2 + hh
                            nc.vector.tensor_tensor(
                                vp[pr][:kk, c0 + hh * 65 : c0 + hh * 65 + 64],
                                pv[:kk, h * 64 : (h + 1) * 64],
                                bv_bc[:kk, h * 64 : (h + 1) * 64],
                                op=ALU.add,
                            )

                # q^T [hd, queries] (queries = tokens 0..2047)
                for i in range(2):
                    for qc in range(NQC):
                        pq = psb.tile([128, QCH], F32, tag="pq")
                        for dd in range(8):
                            nc.tensor.matmul(
                                pq[:],
                                w_sb["q"][:, dd * GD + i * 128 : dd * GD + i * 128 + 128],
                                xt[dd][:, qc * QCH : (qc + 1) * QCH],
                                start=(dd == 0),
                                stop=(dd == 7),
                            )
                        nc.vector.tensor_scalar_add(
                            qT[i][:, qc * QCH : (qc + 1) * QCH], pq[:], bqt[i][:, 0:1]
                        )
                # k^T [hd, keys] over all T tokens
                for i in range(2):
                    for kc5 in range(5):
                        n = QCH if kc5 < 4 else KREM
                        pk = psb.tile([128, QCH], F32, tag="pq")
                        for dd in range(8):
                            nc.tensor.matmul(
                                pk[:, :n],
                                w_sb["k"][:, dd * GD + i * 128 : dd * GD + i * 128 + 128],
                                xt[dd][:, kc5 * QCH : kc5 * QCH + n],
                                start=(dd == 0),
                                stop=(dd == 7),
                            )
                        nc.vector.tensor_scalar_add(
                            kT[i][:, kc5 * QCH : kc5 * QCH + n], pk[:, :n], bkt[i][:, 0:1]
                        )

            # ---------------- Phase C/D: attention + out-projection ----------------
            with (
                tc.tile_pool(name="sc", bufs=2, space="PSUM") as scp,
                tc.tile_pool(name="av", bufs=2, space="PSUM") as avp,
                tc.tile_pool(name="py", bufs=2, space="PSUM") as pyp,
            ):
                for qc in range(NQC):
                    onTs = []
                    for pr in range(2):
                        VA = avp.tile([128, QCH], F32, tag="av", name=f"VA_{qc}_{pr}")
                        VB = avp.tile([128, QCH], F32, tag="av", name=f"VB_{qc}_{pr}")
                        for kc in range(NKC):
                            kk = 128 if kc < 16 else KREM
                            Sps = scp.tile([128, 2 * QCH], F32, tag="s")
                            PT = ptp.tile([128, 2 * QCH], F32R, tag="pt")
                            nc.tensor.matmul(
                                Sps[:kk, 0:QCH],
                                kT[pr][0:64, kc * 128 : kc * 128 + kk],
                                qT[pr][0:64, qc * QCH : (qc + 1) * QCH],
                                start=True,
                                stop=True,
                            )
                            nc.tensor.matmul(
                                Sps[:kk, QCH : 2 * QCH],
                                kT[pr][64:128, kc * 128 : kc * 128 + kk],
                                qT[pr][64:128, qc * QCH : (qc + 1) * QCH],
                                start=True,
                                stop=True,
                            )
                            nc.scalar.activation(PT[:kk, :], Sps[:kk, :], AF.Exp)
                            st, stp = kc == 0, kc == NKC - 1
                            c0 = kc * VBLK
                            nc.tensor.matmul(
                                VA[0:65, :], vp[pr][:kk, c0 : c0 + 65],
                                PT[:kk, 0:QCH], start=st, stop=stp,
                            )
                            nc.tensor.matmul(
                                VB[0:65, :], vp[pr][:kk, c0 + 65 : c0 + VBLK],
                                PT[:kk, QCH : 2 * QCH], start=st, stop=stp,
                            )
                        onT = onp.tile([128, QCH], F32R, tag="onT")
                        for hh, V in ((0, VA), (1, VB)):
                            rc = sp.tile([1, QCH], F32, tag="rc")
                            nc.vector.reciprocal(rc[:], V[64:65, :])
                            bcs = sp.tile([64, QCH], F32, tag="bcs")
                            nc.gpsimd.partition_broadcast(bcs[:], rc[:])
                            nc.vector.tensor_tensor(
                                onT[hh * 64 : hh * 64 + 64, :],
                                V[0:64, :], bcs[:], op=ALU.mult,
                            )
                        onTs.append(onT)
                    for ts_ in range(4):
                        for dc in range(2):
                            py = pyp.tile([128, QCH], F32, tag="py")
                            nc.tensor.matmul(
                                py[:], onTs[0][:, ts_ * 128 : (ts_ + 1) * 128],
                                wo_sb[0][:, dc * QCH : (dc + 1) * QCH],
                                start=True, stop=False,
                            )
                            nc.tensor.matmul(
                                py[:], onTs[1][:, ts_ * 128 : (ts_ + 1) * 128],
                                wo_sb[1][:, dc * QCH : (dc + 1) * QCH],
                                start=False, stop=True,
                            )
                            ys = yp.tile([128, QCH], F32, tag="ys")
                            nc.vector.tensor_tensor(
                                ys[:], py[:], bo_bc[:, dc * QCH : (dc + 1) * QCH],
                                op=ALU.add,
                            )
                            nc.sync.dma_start(
                                y_d[qc * QCH + ts_ * 128 : qc * QCH + (ts_ + 1) * 128,
                                    dc * QCH : (dc + 1) * QCH],
                                ys[:],
                            )
    nc.compile()
    return nc


def _get_program():
    if "nc" not in _prog_cache:
        _prog_cache["nc"] = _build_program()
    return _prog_cache["nc"]


def _host_inputs(x, sink_tokens, wq, bq, wk, bk, wv, bv, wo, bo):
    f = np.float32
    x = np.asarray(x, f)
    sink = np.asarray(sink_tokens, f)[0]            # [NS, D]
    wq, wk, wv, wo = (np.asarray(a, f) for a in (wq, wk, wv, wo))
    bq, bk, bv, bo = (np.asarray(a, f) for a in (bq, bk, bv, bo))
    sc = np.float32(1.0 / np.sqrt(HD))
    in_maps = []
    for core in range(NCORES):
        b, g = core // 4, core % 4
        xs = np.concatenate([x[b], sink], axis=0)   # sinks LAST
        xT = np.ascontiguousarray(xs.T)
        sl = slice(g * GD, (g + 1) * GD)
        in_maps.append({
            "xT": xT,
            "wq_t": np.ascontiguousarray(wq[sl].T) * sc,
            "wk_t": np.ascontiguousarray(wk[sl].T),
            "wv_t": np.ascontiguousarray(wv[sl].T),
            "wo_t": np.ascontiguousarray(wo[:, sl].T),
            "bq": (bq[sl] * sc).reshape(GD, 1).copy(),
            "bk": bk[sl].reshape(GD, 1).copy(),
            "bv": bv[sl].reshape(1, GD).copy(),
            "bo": (bo if g == 0 else np.zeros_like(bo)).reshape(1, D).copy(),
        })
    return in_maps


def kernel(x, sink_tokens, wq, bq, wk, bk, wv, bv, wo, bo):
    from concourse.bass_utils import run_bass_kernel_spmd

    nc = _get_program()
    in_maps = _host_inputs(x, sink_tokens, wq, bq, wk, bk, wv, bv, wo, bo)
    res = run_bass_kernel_spmd(nc, in_maps, core_ids=list(range(NCORES)))
    y = np.zeros((B, S, D), np.float64)
    for core in range(NCORES):
        y[core // 4] += res.results[core]["y"]
    return y.astype(np.float32)
